# revision 15
# baseline (speedup 1.0000x reference)
"""Trainium2 Bass kernel for nn_CNN3_P (dense_cnn), 8-core data parallel.

Network (per sample):
  x [128,64] -> pairwise conv -> relu -> [256,127]
  -> conv1d k3 (x3, relu) -> [256,121] -> FC 30976->512 relu -> FC 512->1

Strategy: batch 2048 split 256/core. Channels on partitions (2 chunks of
128); all layers run on a flat [128, T*128] layout (stride 128 per
sample) where the K=3 conv shifts are plain column offsets; boundary
columns hold garbage that never reaches valid outputs. All matmuls in
fp16 (1 cyc/row on the PE; fp32 operands stream at half rate), PSUM
accumulates fp32. Conv3 output is stored (l, s)-major so FC1's
stationary operands are contiguous; Wf1 streams through SBUF once.
"""
import os
import sys

for _p in ('/opt/trn_rl_repo', '/root/.axon_site/_ro/trn_rl_repo'):
    if os.path.isdir(_p) and _p not in sys.path:
        sys.path.insert(0, _p)

import numpy as np
import ml_dtypes

import concourse.bacc as bacc
import concourse.mybir as mybir
import concourse.tile as tile
from concourse.bass_utils import run_bass_kernel_spmd
from concourse.masks import make_identity

F32 = mybir.dt.float32
F16 = mybir.dt.float16

P = 128
CL = 128          # context length
IL = 64           # inst length
PC = 256          # channels (all layers)
NCHUNK = 2        # channel chunks of 128
LF = 121          # conv3 valid positions
F1 = 512
N_CORES = 8
B = 2048
BCORE = B // N_CORES      # 256
T = 8                     # samples per conv sub-tile
NT = BCORE // T           # 32
FLAT = T * CL             # 1024
TILE_N = 512              # psum tile width (4 samples * 128)
NTC = FLAT // TILE_N      # 2
SPT = TILE_N // CL        # samples per psum tile (4)
SC = BCORE // P           # 2 sample chunks of 128 for FC


def build_nc():
    nc = bacc.Bacc("TRN2", target_bir_lowering=False, debug=False)

    x_d = nc.dram_tensor("x", [BCORE, CL * IL], F16, kind="ExternalInput")
    wp1_d = nc.dram_tensor("wp1t", [IL, PC], F16, kind="ExternalInput")
    wp0_d = nc.dram_tensor("wp0t", [IL, PC], F16, kind="ExternalInput")
    bp_d = nc.dram_tensor("bpc", [NCHUNK, P], F32, kind="ExternalInput")
    wc_d = [nc.dram_tensor(f"w{i}t", [NCHUNK, 3, NCHUNK, P, P], F16,
                           kind="ExternalInput") for i in (1, 2, 3)]
    bc_d = [nc.dram_tensor(f"b{i}c", [NCHUNK, P], F32, kind="ExternalInput")
            for i in (1, 2, 3)]
    wf1_d = nc.dram_tensor("wf1t", [PC, LF, F1], F16, kind="ExternalInput")
    bf1_d = nc.dram_tensor("bf1r", [1, F1], F16, kind="ExternalInput")
    wf2_d = nc.dram_tensor("wf2p", [4, P, P], F16, kind="ExternalInput")
    bf2_d = nc.dram_tensor("bf2s", [1, 1], F32, kind="ExternalInput")
    ones_d = nc.dram_tensor("onesr", [1, P], F16, kind="ExternalInput")
    y_d = nc.dram_tensor("y", [BCORE, 1], F32, kind="ExternalOutput")

    RELU = mybir.ActivationFunctionType.Relu

    with tile.TileContext(nc) as tc:
        with tc.tile_pool(name="const", bufs=1) as cpool, \
             tc.tile_pool(name="h3c", bufs=1) as h3pool:
            # --- constants / weights, resident all kernel ---
            ident = cpool.tile([P, P], F16)
            make_identity(nc, ident[:])
            wp1 = cpool.tile([IL, PC], F16)
            nc.sync.dma_start(wp1[:], wp1_d.ap())
            wp0 = cpool.tile([IL, PC], F16)
            nc.sync.dma_start(wp0[:], wp0_d.ap())
            bp = cpool.tile([P, NCHUNK], F32)
            nc.sync.dma_start(bp[:], bp_d.ap().rearrange("c p -> p c"))
            # conv weights: per layer, per ci-chunk: [ci, (k, coc, co)]
            wconv = []
            for i in range(3):
                tiles = []
                for cic in range(NCHUNK):
                    w = cpool.tile([P, 3 * NCHUNK * P], F16, tag=f"w{i}_{cic}")
                    nc.sync.dma_start(
                        w[:].rearrange("p (k b c) -> p k b c", k=3, b=NCHUNK),
                        wc_d[i].ap()[cic].rearrange("k b p c -> p k b c"))
                    tiles.append(w)
                wconv.append(tiles)
            bconv = []
            for i in range(3):
                bt = cpool.tile([P, NCHUNK], F32, tag=f"bc{i}")
                nc.sync.dma_start(bt[:], bc_d[i].ap().rearrange("c p -> p c"))
                bconv.append(bt)
            bf1 = cpool.tile([1, F1], F16)
            nc.sync.dma_start(bf1[:], bf1_d.ap())
            wf2 = cpool.tile([P, 4 * P], F16)
            nc.sync.dma_start(wf2[:].rearrange("p (f m) -> p f m", f=4),
                              wf2_d.ap().rearrange("f p m -> p f m"))
            bf2 = cpool.tile([1, 1], F32)
            nc.sync.dma_start(bf2[:], bf2_d.ap())
            ones = cpool.tile([1, P], F16)
            nc.sync.dma_start(ones[:], ones_d.ap())

            # persistent conv3 output, fp16, (l, s)-major: col = l*BCORE + s
            h3c = [h3pool.tile([P, CL * BCORE], F16, tag=f"h3c{cc}", name=f"h3c{cc}")
                   for cc in range(NCHUNK)]
            h3v = [h.rearrange("p (l s) -> p l s", s=BCORE) for h in h3c]

            # ---------------- conv phase ----------------
            with tc.tile_pool(name="xn", bufs=2) as xnpool, \
                 tc.tile_pool(name="xt", bufs=2) as xtpool, \
                 tc.tile_pool(name="h", bufs=2) as hpool, \
                 tc.tile_pool(name="ps", bufs=8, space="PSUM") as pspool:
                xv = x_d.ap().rearrange("b (i j) -> i b j", j=IL)
                for t in range(NT):
                    xn = xnpool.tile([P, T * IL], F16, tag="xn")
                    nc.sync.dma_start(
                        xn[:].rearrange("p (b j) -> p b j", j=IL),
                        xv[:, t * T:(t + 1) * T, :])
                    xt = xtpool.tile([IL, T * CL], F16, tag="xt")
                    xb = xtpool.tile([IL, T * CL], F16, tag="xb")
                    for pr in range(T // 2):
                        tp = pspool.tile([P, P], F16, tag="ps")
                        nc.tensor.transpose(tp[:], xn[:, pr * P:(pr + 1) * P], ident[:])
                        nc.vector.tensor_copy(
                            xt[:, (2 * pr) * CL:(2 * pr + 1) * CL], tp[0:IL, :])
                        nc.vector.tensor_copy(
                            xt[:, (2 * pr + 1) * CL:(2 * pr + 2) * CL], tp[IL:P, :])
                    # broadcast x0 column across each sample's block
                    for s in range(T):
                        nc.vector.tensor_copy(
                            xb[:, s * CL:(s + 1) * CL],
                            xt[:, s * CL:s * CL + 1].broadcast_to([IL, CL]))
                    # pairwise: h0[c, (s,i)] = relu(Wp1@xt[:,i] + Wp0@x0 + bp)
                    h0 = [hpool.tile([P, FLAT], F16, tag=f"h0_{cc}", name=f"h0_{cc}")
                          for cc in range(NCHUNK)]
                    for cc in range(NCHUNK):
                        for nt in range(NTC):
                            ps = pspool.tile([P, TILE_N], F32, tag="ps")
                            sl_ = slice(nt * TILE_N, (nt + 1) * TILE_N)
                            nc.tensor.matmul(ps[:], wp1[:, cc * P:(cc + 1) * P],
                                             xt[:, sl_], start=True, stop=False)
                            nc.tensor.matmul(ps[:], wp0[:, cc * P:(cc + 1) * P],
                                             xb[:, sl_], start=False, stop=True)
                            nc.scalar.activation(h0[cc][:, sl_], ps[:],
                                                 RELU, bias=bp[:, cc:cc + 1])

                    def conv_layer(hin, w_tiles, evac):
                        pss = {}
                        for co in range(NCHUNK):
                            for nt in range(NTC):
                                pss[co, nt] = pspool.tile([P, TILE_N], F32,
                                                          tag="ps", name=f"cps{co}_{nt}")
                        step = 0
                        for k in range(3):
                            for ci in range(NCHUNK):
                                for co in range(NCHUNK):
                                    lhsT = w_tiles[ci][:, (k * NCHUNK + co) * P:
                                                       (k * NCHUNK + co + 1) * P]
                                    for nt in range(NTC):
                                        nk = min(TILE_N, FLAT - nt * TILE_N - k)
                                        nc.tensor.matmul(
                                            pss[co, nt][:, 0:nk], lhsT,
                                            hin[ci][:, nt * TILE_N + k:
                                                    nt * TILE_N + k + nk],
                                            start=(step == 0), stop=(step == 5))
                                step += 1
                        for co in range(NCHUNK):
                            for nt in range(NTC):
                                evac(co, nt, pss[co, nt])

                    h1 = [hpool.tile([P, FLAT], F16, tag=f"h1_{cc}", name=f"h1_{cc}")
                          for cc in range(NCHUNK)]

                    def evac1(co, nt, ps):
                        nc.scalar.activation(h1[co][:, nt * TILE_N:(nt + 1) * TILE_N],
                                             ps[:], RELU, bias=bconv[0][:, co:co + 1])
                    conv_layer(h0, wconv[0], evac1)

                    h2 = [hpool.tile([P, FLAT], F16, tag=f"h2_{cc}", name=f"h2_{cc}")
                          for cc in range(NCHUNK)]

                    def evac2(co, nt, ps):
                        nc.scalar.activation(h2[co][:, nt * TILE_N:(nt + 1) * TILE_N],
                                             ps[:], RELU, bias=bconv[1][:, co:co + 1])
                    conv_layer(h1, wconv[1], evac2)

                    def evac3(co, nt, ps):
                        s0 = t * T + nt * SPT
                        nc.scalar.activation(
                            h3v[co][:, :, s0:s0 + SPT].rearrange("p l s -> p s l"),
                            ps[:].rearrange("p (s l) -> p s l", l=CL),
                            RELU, bias=bconv[2][:, co:co + 1])
                    conv_layer(h2, wconv[2], evac3)

            # ---------------- FC phase ----------------
            with tc.tile_pool(name="wf1", bufs=16) as wfpool, \
                 tc.tile_pool(name="h4", bufs=1) as h4pool, \
                 tc.tile_pool(name="fps", bufs=2, space="PSUM") as fpspool:
                ps_fc1 = [fpspool.tile([P, F1], F32, tag="fc1ps", name=f"fc1ps{sc}")
                          for sc in range(SC)]
                for sc in range(SC):
                    nc.tensor.matmul(ps_fc1[sc][:], ones[:], bf1[:],
                                     start=True, stop=False)
                for l in range(LF):
                    for cc in range(NCHUNK):
                        rw = wfpool.tile([P, F1], F16, tag="wf1")
                        nc.sync.dma_start(rw[:], wf1_d.ap()[cc * P:(cc + 1) * P, l, :])
                        last = (l == LF - 1) and (cc == NCHUNK - 1)
                        for sc in range(SC):
                            # valid conv3 position l sits at flat l+1
                            nc.tensor.matmul(ps_fc1[sc][:],
                                             h3v[cc][:, l + 1, sc * P:(sc + 1) * P],
                                             rw[:], start=False, stop=last)
                h4 = []
                for sc in range(SC):
                    h = h4pool.tile([P, F1], F16, tag=f"h4_{sc}", name=f"h4_{sc}")
                    nc.scalar.activation(h[:], ps_fc1[sc][:], RELU)
                    h4.append(h)
                # FC2: transpose h4 then contract f on partitions
                ystage = h4pool.tile([1, BCORE], F32, tag="ystage")
                for sc in range(SC):
                    h4t = h4pool.tile([P, 4 * P], F16, tag=f"h4t_{sc}",
                                      name=f"h4t_{sc}")
                    for fc in range(4):
                        tp = fpspool.tile([P, P], F16, tag="fc2tp")
                        nc.tensor.transpose(tp[:], h4[sc][:, fc * P:(fc + 1) * P],
                                            ident[:])
                        nc.vector.tensor_copy(h4t[:, fc * P:(fc + 1) * P], tp[:])
                    po = fpspool.tile([P, P], F32, tag="fc2ps")
                    for fc in range(4):
                        nc.tensor.matmul(po[:], wf2[:, fc * P:(fc + 1) * P],
                                         h4t[:, fc * P:(fc + 1) * P],
                                         start=(fc == 0), stop=(fc == 3))
                    nc.vector.tensor_scalar_add(ystage[:, sc * P:(sc + 1) * P],
                                                po[0:1, :], bf2[:])
                nc.sync.dma_start(y_d.ap().rearrange("b one -> one b"), ystage[:])

    nc.compile()
    return nc


_NC_CACHE = None


def _get_nc():
    global _NC_CACHE
    if _NC_CACHE is None:
        _NC_CACHE = build_nc()
    return _NC_CACHE


def prep_inputs(x, Wp, bp, W1, b1, W2, b2, W3, b3, Wf1, bf1, Wf2, bf2):
    """Host-side shard + weight re-layout. Returns per-core input maps."""
    f32, f16 = np.float32, np.float16
    wp1t = np.ascontiguousarray(np.asarray(Wp, f32)[:, :, 1].T).astype(f16)
    wp0t = np.ascontiguousarray(np.asarray(Wp, f32)[:, :, 0].T).astype(f16)
    bpc = np.ascontiguousarray(np.asarray(bp, f32).reshape(NCHUNK, P))

    def conv_t(W):
        # W [co, ci, k] -> [cic, k, coc, ci, co]
        a = np.asarray(W, f32).reshape(NCHUNK, P, NCHUNK, P, 3)
        return np.ascontiguousarray(a.transpose(2, 4, 0, 3, 1)).astype(f16)

    w1t, w2t, w3t = conv_t(W1), conv_t(W2), conv_t(W3)
    b1c = np.ascontiguousarray(np.asarray(b1, f32).reshape(NCHUNK, P))
    b2c = np.ascontiguousarray(np.asarray(b2, f32).reshape(NCHUNK, P))
    b3c = np.ascontiguousarray(np.asarray(b3, f32).reshape(NCHUNK, P))
    # Wf1 [512, 30976] -> [c, l, f] fp16
    wf1t = np.ascontiguousarray(
        np.asarray(Wf1, f32).reshape(F1, PC, LF).transpose(1, 2, 0)).astype(f16)
    bf1r = np.ascontiguousarray(np.asarray(bf1, f32).reshape(1, F1)).astype(f16)
    wf2p = np.zeros((4, P, P), f16)
    wf2p[:, :, 0] = np.asarray(Wf2, f32).reshape(4, P)
    bf2s = np.asarray(bf2, f32).reshape(1, 1)
    onesr = np.ones((1, P), f16)

    shared = dict(wp1t=wp1t, wp0t=wp0t, bpc=bpc, w1t=w1t, w2t=w2t, w3t=w3t,
                  b1c=b1c, b2c=b2c, b3c=b3c, wf1t=wf1t, bf1r=bf1r,
                  wf2p=wf2p, bf2s=bf2s, onesr=onesr)
    xs = np.asarray(x, f32).reshape(N_CORES, BCORE, CL * IL).astype(f16)
    return [dict(x=np.ascontiguousarray(xs[i]), **shared) for i in range(N_CORES)]


def kernel(x, Wp, bp, W1, b1, W2, b2, W3, b3, Wf1, bf1, Wf2, bf2,
           trace=False, **run_kwargs):
    nc = _get_nc()
    in_maps = prep_inputs(x, Wp, bp, W1, b1, W2, b2, W3, b3, Wf1, bf1, Wf2, bf2)
    res = run_bass_kernel_spmd(nc, in_maps, core_ids=list(range(N_CORES)),
                               trace=trace, **run_kwargs)
    out = np.concatenate([res.results[i]["y"] for i in range(N_CORES)], axis=0)
    kernel.last_results = res
    return out.astype(np.float32)


kernel.last_results = None
